# revision 2
# baseline (speedup 1.0000x reference)
"""DeformableConv Trainium2 Bass kernel, v2.

B=8, Cin=128, Cout=256, H=W=64, K=3. Data-parallel over batch: core b
processes sample b. Per-core pipeline:

  1. offset conv (PE, 9 shifted-AP matmuls on a 72x72 zero-padded bf16
     image) -> offsets [18, 4096]; PE-transpose to pixel-major.
  2. coords/weights on DVE in pixel-major [128, (32 pt, 9 tap)] layout:
     r0/c0 = floor(py/px), wy/wx fractions, wxy = wy*wx, flat cell id.
  3. derivative planes Dy/Dx/Dxy of the padded image (DVE subs); PE
     transposes all 4 planes to cell-major and packs an HBM image
     himg[cell, 4*128] where each 1KB row holds [x, Dy, Dx, Dxy][c]
     for one cell.  Bilinear then is v = x + wy*Dy + wx*Dx + wxy*Dxy.
  4. dma_gather (DMA engines, not gpsimd ucode) fetches one 1KB row per
     (tap, pixel) into pixel-major tiles G[pix, tap, plane*128+c].
  5. per (pixel-tile, tap): 3 fused scalar_tensor_tensor MACs on DVE
     with per-partition (=per-pixel) scalars -> vals[pix, c]; PE
     back-transpose -> [c, pix]; PE matmuls accumulate the 9-tap conv
     into PSUM with deform_w stationary.
"""

import sys

sys.path.insert(0, "/opt/trn_rl_repo")

import numpy as np
import ml_dtypes

import concourse.bass as bass
import concourse.tile as tile
from concourse import bacc, mybir
from concourse.bass_utils import run_bass_kernel_spmd
from contextlib import ExitStack

F32 = mybir.dt.float32
BF16 = mybir.dt.bfloat16
I16 = mybir.dt.int16
I32 = mybir.dt.int32
ALU = mybir.AluOpType

B, CIN, COUT, H, W = 8, 128, 256, 64, 64
K2 = 9
HW = H * W                  # 4096
PADW = 72                   # padded image 72x72, origin shift +3
FLAT = PADW * PADW          # 5184
NCELL = 5376                # 42 chunks of 128 cells (rows 72..74 are pad)
XLEN = NCELL + 128          # slack so shifted reads stay in-bounds
NPT = 32                    # pixel tiles of 128
NBLK = 8                    # gather blocks of 4 pixel tiles
PTB = NPT // NBLK           # 4
NIDX = K2 * PTB * 128       # 4608 indices per gather

_cache = {}


def _build_program(num_devices=B):
    nc = bacc.Bacc("TRN2", target_bir_lowering=False, debug=False,
                   num_devices=num_devices)

    xbf_ext = nc.declare_dram_parameter("xbf", [CIN, HW], BF16, isOutput=False)
    woff_ext = nc.declare_dram_parameter("woff", [CIN, K2, 18], BF16, isOutput=False)
    wr_ext = nc.declare_dram_parameter("wr", [CIN, K2, 2, 128], BF16, isOutput=False)
    idb_ext = nc.declare_dram_parameter("idb", [128, 128], BF16, isOutput=False)
    idf_ext = nc.declare_dram_parameter("idf", [18, 18], F32, isOutput=False)
    ybk_ext = nc.declare_dram_parameter("ybk", [128, 288], F32, isOutput=False)
    xbk_ext = nc.declare_dram_parameter("xbk", [128, 288], F32, isOutput=False)
    out_ext = nc.declare_dram_parameter("out", [2, 128, HW], BF16, isOutput=True)

    himg = nc.dram_tensor("himg", [NCELL, 512], BF16)
    psem = nc.alloc_semaphore("psem")

    with tile.TileContext(nc) as tc:
        with ExitStack() as ctx:
            sb = ctx.enter_context(tc.tile_pool(name="sb", bufs=1))
            sbc = ctx.enter_context(tc.tile_pool(name="sbc", bufs=1))
            phase1 = ExitStack()
            sbp = phase1.enter_context(tc.tile_pool(name="sbp", bufs=1))
            stg = phase1.enter_context(tc.tile_pool(name="stg", bufs=2))
            ppk = ctx.enter_context(tc.tile_pool(name="ppk", bufs=1, space="PSUM"))
            tpool = ctx.enter_context(tc.tile_pool(name="tp", bufs=2, space="PSUM"))
            opool = ctx.enter_context(tc.tile_pool(name="op", bufs=2, space="PSUM"))

            # ---- constants to SBUF ----
            woff = sb.tile([CIN, K2, 18], BF16)
            nc.gpsimd.dma_start(out=woff[:, :, :], in_=woff_ext[:, :, :])
            wr = sb.tile([CIN, K2, 2, 128], BF16)
            nc.gpsimd.dma_start(out=wr[:, :, :, :], in_=wr_ext[:, :, :, :])
            idb = sb.tile([128, 128], BF16)
            nc.gpsimd.dma_start(out=idb[:, :], in_=idb_ext[:, :])
            idf = sb.tile([18, 18], F32)
            nc.gpsimd.dma_start(out=idf[:, :], in_=idf_ext[:, :])
            ybk = sb.tile([128, 288], F32)
            nc.gpsimd.dma_start(out=ybk[:, :], in_=ybk_ext[:, :])
            xbk = sb.tile([128, 288], F32)
            nc.gpsimd.dma_start(out=xbk[:, :], in_=xbk_ext[:, :])

            # ---- padded bf16 image ----
            xpad = sbp.tile([CIN, XLEN], BF16)
            nc.vector.memset(xpad[:, :], 0.0)
            xbfs = sb.tile([CIN, HW], BF16)
            nc.gpsimd.dma_start(out=xbfs[:, :], in_=xbf_ext[:, :])
            img72 = xpad[:, :FLAT].rearrange("c (r q) -> c r q", r=PADW)
            nc.vector.tensor_copy(
                img72[:, 3:67, 3:67],
                xbfs[:, :].rearrange("c (r q) -> c r q", r=H),
            )

            # ---- offset conv -> off [18, 4096] fp32 ----
            off = sbc.tile([18, HW], F32)
            for t8 in range(8):
                y0 = t8 * 8
                po = ppk.tile([18, 512], F32, tag="poff")
                for t in range(K2):
                    ky, kx = t // 3 - 1, t % 3 - 1
                    rhs = img72[:, y0 + ky + 3 : y0 + ky + 11, kx + 3 : kx + 67]
                    nc.tensor.matmul(po[:, :], woff[:, t, :], rhs,
                                     start=(t == 0), stop=(t == K2 - 1))
                nc.scalar.copy(off[:, t8 * 512 : (t8 + 1) * 512], po[:, :])

            # ---- transpose offsets -> offT [128, (32 pt, 9 k, 2)] fp32 ----
            offT = sbc.tile([128, NPT, K2, 2], F32)
            for pt in range(NPT):
                pot = ppk.tile([128, 18], F32, tag="poff")
                nc.tensor.transpose(out=pot[:, :], in_=off[:, pt * 128 : (pt + 1) * 128],
                                    identity=idf[:, :])
                nc.scalar.copy(offT[:, pt, :, :], pot[:, :])

            # ---- coords / weights / cell index ----
            def cwt(name):
                return sbc.tile([128, 288], F32, tag=name, name=name)

            oy = offT[:, :, :, 0].rearrange("p a b -> p (a b)")
            ox = offT[:, :, :, 1].rearrange("p a b -> p (a b)")
            py = cwt("py")
            nc.vector.tensor_tensor(py[:, :], oy, ybk[:, :], op=ALU.add)
            px = cwt("px")
            nc.vector.tensor_tensor(px[:, :], ox, xbk[:, :], op=ALU.add)
            pyc = cwt("pyc")
            nc.vector.tensor_scalar(pyc[:, :], py[:, :], -2.99, 65.99, op0=ALU.max, op1=ALU.min)
            pxc = cwt("pxc")
            nc.vector.tensor_scalar(pxc[:, :], px[:, :], -2.99, 65.99, op0=ALU.max, op1=ALU.min)
            py, px = pyc, pxc
            # robust floor: works for both truncating and rounding f32->i32
            def floorv(src, pref):
                ti = sbc.tile([128, 288], I32, tag="fvi", name=pref + "i")
                nc.vector.tensor_copy(ti[:, :], src[:, :])
                tf = sbc.tile([128, 288], F32, tag="fvf", name=pref + "f")
                nc.vector.tensor_copy(tf[:, :], ti[:, :])
                neg = sbc.tile([128, 288], F32, tag="fvn", name=pref + "n")
                nc.vector.tensor_tensor(neg[:, :], src[:, :], tf[:, :], op=ALU.subtract)
                nc.vector.tensor_scalar(neg[:, :], neg[:, :], 0.0, None, op0=ALU.is_lt)
                fo = cwt(pref + "0")
                nc.vector.tensor_tensor(fo[:, :], tf[:, :], neg[:, :], op=ALU.subtract)
                return fo

            r0 = floorv(py, "r")
            wy = cwt("wy")
            nc.vector.tensor_tensor(wy[:, :], py[:, :], r0[:, :], op=ALU.subtract)
            c0 = floorv(px, "c")
            wx = cwt("wx")
            nc.vector.tensor_tensor(wx[:, :], px[:, :], c0[:, :], op=ALU.subtract)
            wxy = cwt("wxy")
            nc.vector.tensor_tensor(wxy[:, :], wy[:, :], wx[:, :], op=ALU.mult)
            # flat cell id = (r0+3)*72 + (c0+3)
            fl = cwt("fl")
            nc.vector.scalar_tensor_tensor(fl[:, :], r0[:, :], 72.0, c0[:, :],
                                           ALU.mult, ALU.add)
            pfi = sbc.tile([128, NPT, K2], I16, tag="pfi", name="pfi")
            nc.vector.tensor_scalar(
                pfi[:, :, :], fl[:, :].rearrange("p (t k) -> p t k", t=NPT),
                219.0, None, op0=ALU.add)

            # ---- wrapped gather indices WI[128, pt, k, sub] ----
            # gather j for block b enumerates (lpt, t, p): j = (lpt*9+t)*128+p,
            # so wrap slot j//16 = (lpt*9+t)*8 + p//16 -> free order (pt, k, sub)
            WI = sb.tile([128, NPT, K2, 8], I16)
            for sub in range(8):
                nc.gpsimd.dma_start(
                    out=WI[0:16, :, :, sub],
                    in_=pfi[sub * 16 : sub * 16 + 16, :, :],
                )
            for rep in [16, 32, 64]:
                nc.gpsimd.dma_start(out=WI[rep : 2 * rep, :, :, :],
                                    in_=WI[0:rep, :, :, :])

            # ---- derivative planes (bf16) ----
            dyp = sbp.tile([CIN, XLEN], BF16, name="dyp")
            dxp = sbp.tile([CIN, XLEN], BF16, name="dxp")
            dxyp = sbp.tile([CIN, XLEN], BF16, name="dxyp")
            nc.vector.memset(dyp[:, FLAT:], 0.0)
            nc.vector.memset(dxp[:, FLAT + PADW :], 0.0)
            nc.vector.memset(dxyp[:, FLAT:], 0.0)
            nc.vector.tensor_tensor(dyp[:, :FLAT], xpad[:, PADW : FLAT + PADW],
                                    xpad[:, :FLAT], op=ALU.subtract)
            nc.vector.tensor_tensor(dxp[:, : FLAT + PADW], xpad[:, 1 : FLAT + PADW + 1],
                                    xpad[:, : FLAT + PADW], op=ALU.subtract)
            nc.vector.tensor_tensor(dxyp[:, :FLAT], dxp[:, PADW : FLAT + PADW],
                                    dxp[:, :FLAT], op=ALU.subtract)

            # ---- pack planes to HBM: himg[cell, (plane, c)] ----
            planes = [xpad, dyp, dxp, dxyp]
            for w in range(7):
                stgt = stg.tile([128, 6, 512], BF16, tag="stg")
                if w >= 2:
                    # stgt reuses the buffer the pack-DMA of round w-2 read;
                    # that DMA's completion is only visible through psem.
                    nc.scalar.wait_ge(psem, (w - 1) * 16)
                for j in range(6):
                    ch = w * 6 + j
                    pT = ppk.tile([128, 512], BF16, tag="pT")
                    for pi in range(4):
                        nc.tensor.transpose(
                            out=pT[:, pi * 128 : (pi + 1) * 128],
                            in_=planes[pi][:, ch * 128 : (ch + 1) * 128],
                            identity=idb[:, :])
                    nc.scalar.copy(stgt[:, j, :], pT[:, :])
                nc.gpsimd.dma_start(
                    out=himg[w * 768 : (w + 1) * 768, :].rearrange(
                        "(j p) e -> p j e", p=128),
                    in_=stgt[:, :, :],
                ).then_inc(psem, 16)

            # ---- release plane/staging SBUF, open gather-phase pools ----
            phase1.close()
            gpool = ctx.enter_context(tc.tile_pool(name="gp", bufs=3))
            vpool = ctx.enter_context(tc.tile_pool(name="vp", bufs=2))
            spool = ctx.enter_context(tc.tile_pool(name="sp", bufs=2))
            otp = ctx.enter_context(tc.tile_pool(name="otp", bufs=2))

            # ---- main loop: gather + weight + conv ----
            import os as _os
            STAGE = int(_os.environ.get("DEFORM_STAGE", "0"))
            if STAGE == 1:
                nc.gpsimd.dma_start(out=out_ext[0, 0:18, :], in_=off[:, :])
                pfif = sbc.tile([128, 288], F32, tag="pfif", name="pfif")
                nc.vector.tensor_copy(pfif[:, :],
                                      pfi[:, :, :].rearrange("p a b -> p (a b)"))
                nc.gpsimd.dma_start(out=out_ext[1, :, 0:288], in_=pfif[:, :])
                nc.gpsimd.dma_start(out=out_ext[1, :, 288:576], in_=wy[:, :])
                nc.gpsimd.dma_start(out=out_ext[1, :, 576:864], in_=wx[:, :])
                nc.gpsimd.dma_start(out=out_ext[1, :, 864:1152], in_=wxy[:, :])
            nc.gpsimd.wait_ge(psem, 7 * 16)
            if STAGE == 2:
                # dump gathered planes of block 0
                G0 = gpool.tile([128, K2 * PTB, 512], BF16, tag="G")
                nc.gpsimd.dma_gather(
                    G0[:, :, :], himg[:, :], WI[:, 0:PTB, :, :],
                    num_idxs=NIDX, num_idxs_reg=NIDX, elem_size=512,
                    single_packet=False)
                gd = sbc.tile([128, 2048], F32, tag="gd", name="gd")
                nc.vector.tensor_copy(gd[:, :],
                                      G0[:, 0:4, :].rearrange("p a b -> p (a b)"))
                nc.gpsimd.dma_start(out=out_ext[0, :, 0:2048], in_=gd[:, :])
            Gs = {}

            def issue_gather(bb):
                G = gpool.tile([128, K2 * PTB, 512], BF16, tag="G", name="G")
                nc.gpsimd.dma_gather(
                    G[:, :, :], himg[:, :], WI[:, bb * PTB : (bb + 1) * PTB, :, :],
                    num_idxs=NIDX, num_idxs_reg=NIDX, elem_size=512,
                    single_packet=False)
                Gs[bb] = G

            nblk_run = NBLK if STAGE == 0 else 0
            for bb in range(min(2, nblk_run)):
                issue_gather(bb)
            for b in range(nblk_run):
                if b + 2 < nblk_run:
                    issue_gather(b + 2)
                G = Gs.pop(b)
                vsb = spool.tile([128, K2, 512], BF16, tag="vsb")
                for t in range(K2):
                    ptb = tpool.tile([128, 512], BF16, tag="ptb")
                    for l in range(PTB):
                        sl = l * K2 + t
                        cw = (b * PTB + l) * K2 + t
                        vals = vpool.tile([128, 128], BF16, tag=f"v{l}")
                        eng = nc.vector
                        eng.scalar_tensor_tensor(
                            vals[:, :], G[:, sl, 128:256], wy[:, cw : cw + 1],
                            G[:, sl, 0:128], ALU.mult, ALU.add)
                        eng.scalar_tensor_tensor(
                            vals[:, :], G[:, sl, 256:384], wx[:, cw : cw + 1],
                            vals[:, :], ALU.mult, ALU.add)
                        eng.scalar_tensor_tensor(
                            vals[:, :], G[:, sl, 384:512], wxy[:, cw : cw + 1],
                            vals[:, :], ALU.mult, ALU.add)
                        nc.tensor.transpose(
                            out=ptb[:, l * 128 : (l + 1) * 128],
                            in_=vals[:, :], identity=idb[:, :])
                    nc.scalar.copy(vsb[:, t, :], ptb[:, :])
                ot = otp.tile([128, PTB, 2, 128], BF16, tag="ot")
                for l in range(PTB):
                    for hf in range(2):
                        pso = opool.tile([128, 128], F32, tag="pso", name="pso")
                        for t in range(K2):
                            nc.tensor.matmul(
                                pso[:, :], wr[:, t, hf, :],
                                vsb[:, t, l * 128 : (l + 1) * 128],
                                start=(t == 0), stop=(t == K2 - 1))
                        nc.scalar.copy(ot[:, l, hf, :], pso[:, :])
                for hf in range(2):
                    nc.gpsimd.dma_start(
                        out=out_ext[hf, :, b * 512 : (b + 1) * 512],
                        in_=ot[:, :, hf, :])
    nc.compile()
    return nc


def _prep_consts():
    yb = (np.arange(HW) // W).reshape(NPT, 128).T
    xb = (np.arange(HW) % W).reshape(NPT, 128).T
    ky = np.arange(K2) // 3 - 1
    kx = np.arange(K2) % 3 - 1
    ybk = (yb[:, :, None] + ky[None, None, :]).reshape(128, 288).astype(np.float32)
    xbk = (xb[:, :, None] + kx[None, None, :]).reshape(128, 288).astype(np.float32)
    idb = np.eye(128, dtype=ml_dtypes.bfloat16)
    idf = np.eye(18, dtype=np.float32)
    return ybk, xbk, idb, idf


def _prep_inputs(x, offset_w, offset_b, deform_w):
    ybk, xbk, idb, idf = _prep_consts()
    oby = offset_b.reshape(9, 2)[:, 0]
    obx = offset_b.reshape(9, 2)[:, 1]
    ybk2 = (ybk.reshape(128, 32, 9) + oby[None, None, :]).reshape(128, 288).astype(np.float32)
    xbk2 = (xbk.reshape(128, 32, 9) + obx[None, None, :]).reshape(128, 288).astype(np.float32)
    woff = offset_w.reshape(18, CIN, 3, 3).transpose(1, 2, 3, 0).reshape(CIN, K2, 18)
    woff = np.ascontiguousarray(woff).astype(ml_dtypes.bfloat16)
    wrh = deform_w.reshape(COUT, CIN, K2).transpose(1, 2, 0).reshape(CIN, K2, 2, 128)
    wrh = np.ascontiguousarray(wrh).astype(ml_dtypes.bfloat16)
    consts = {"woff": woff, "wr": wrh, "idb": idb, "idf": idf,
              "ybk": ybk2, "xbk": xbk2}
    in_maps = []
    for bi in range(B):
        m = dict(consts)
        m["xbf"] = np.ascontiguousarray(
            x[bi].reshape(CIN, HW).astype(ml_dtypes.bfloat16))
        in_maps.append(m)
    return in_maps


def kernel(x, offset_w, offset_b, deform_w, deform_b):
    x = np.asarray(x, dtype=np.float32)
    offset_w = np.asarray(offset_w, dtype=np.float32)
    offset_b = np.asarray(offset_b, dtype=np.float32)
    deform_w = np.asarray(deform_w, dtype=np.float32)
    deform_b = np.asarray(deform_b, dtype=np.float32)

    if "nc" not in _cache:
        _cache["nc"] = _build_program()
    nc = _cache["nc"]

    in_maps = _prep_inputs(x, offset_w, offset_b, deform_w)
    res = run_bass_kernel_spmd(nc, in_maps, list(range(B)))
    out = np.stack([r["out"].reshape(COUT, H, W) for r in res.results])
    out = out + deform_b[None, :, None, None]
    return out.astype(np.float32)
